# revision 10
# baseline (speedup 1.0000x reference)
"""CKAFormer Trainium2 kernel: 6 iterations of
    Xn = X / ||X||_row;  P = softmax(relu(Xn@W1+b1)@W2+b2)
    X  = Xn + g*P@(P.T@Xn) - g*Xn@(Xn.T@Xn)
then a final MLP. Row-sharded over 8 NeuronCores.

Fast scheme vs the bf16 baseline (1187us):
- All heavy matmuls (Gram, Xn@G update, P.T@Xn, MLP layer 1) run in
  fp8e4m3 with perf_mode=DoubleRow.  Scales keep everything in fp8's
  normal range: xball/xt8 = 4*Xn, g8 = G, er8 = 64*P, ets8 = 4*P.T,
  ptx8 = -PtX.  GAMMA is applied as a float constant in the
  psum->state scalar_tensor_tensor (out = psum*(-g/4) + Xn).
- The P@PtX term accumulates into the SAME psum group as Xn@G (sign
  folded into the ptx staging), halving the DVE update work.
- X state lives in fp16 (normalized in place, DVE 2x rate) and doubles
  as the 2-byte source for the DMA-XBAR transposes — no PE transposes
  anywhere and no extra bf16 copy of the state.
- Collectives run in fp8: the staging copies write fp8 and the
  AllReduce outputs land directly in the fp8 operand tiles (no casts).
- Gram half 0 iterates row-pairs in the OUTER loop (8 live psum
  groups) so it can start before the previous iteration's norm tail
  has finished all 16 row blocks.
- Final MLP runs in fp16 straight off the fp16 state.
"""

import sys

sys.path.insert(0, "/opt/trn_rl_repo")

import ml_dtypes
import numpy as np

import concourse.bass as bass
import concourse.mybir as mybir
import concourse.tile as tile
from concourse.bass_utils import run_bass_kernel_spmd
from concourse.vector_clock import ScopedClock

DEPTH = 6
GAMMA = 1e-4
DIM = 1024
HIDDEN = 16
OUT_DIM = 64
N = 16384
CORES = 8

NS = N // CORES        # rows per core = 2048
RT = NS // 128         # row tiles = 16
DK = DIM // 128        # dim k-tiles = 8
P = 128
S = 4.0                # fp8 scale on Xn
CG = -GAMMA / S        # psum -> state scale

F32 = mybir.dt.float32
BF = mybir.dt.bfloat16
F16 = mybir.dt.float16
FP8 = mybir.dt.float8e4

# AllReduce staging dtype: fp8 halves the wire bytes and lands directly in
# the fp8 operand tiles; flip to False if the CCE fp8 path misbehaves.
USE_FP8_AR = False
ARDT = FP8 if USE_FP8_AR else BF
AF = mybir.ActivationFunctionType
ALU = mybir.AluOpType
DRM = mybir.MatmulPerfMode.DoubleRow

# this container's walrus only accepts one sync-wait slot per engine
# instruction; hoist excess waits onto preceding EventSemaphore carriers.
_MAX_WAITS = 1


class _TC(tile.TileContext):
    def _drain_and_barrier(self, tick_clock, wait_clock):
        drain_inst = self.nc.sync.drain()
        wait_clock.add_sem_waits(
            drain_inst.ins, ScopedClock({None: tick_clock.global_clock})
        )
        si = drain_inst.ins.sync_info
        w = list(si.on_wait) if si and si.on_wait else []
        if len(w) > 1:
            si.on_wait = w[:1]
            for i in range(1, len(w)):
                c = self.nc.sync.drain()
                c.ins.sync_info = mybir.SyncInfo(on_wait=[w[i]], on_update=[])
        self.nc.all_engine_barrier()
        assert self.sems is not None
        popped = self.nc._tile_sem_poison_stack.pop()
        assert popped is self._sem_poison
        self.nc.clear_and_free_semaphores(list(self.sems.allocated().values()))
        self.nc.all_engine_barrier()


def _split_waits(nc, limit=_MAX_WAITS):
    """Hoist excess sem waits onto EventSemaphore carriers inserted just
    before the over-limit instruction (per-engine program order preserves the
    gating; waits are a conjunction so splitting is sound)."""
    nid = 0
    for bb in nc.main_func.blocks:
        out = []
        changed = False
        for ins in bb.instructions:
            si = ins.sync_info
            w = list(si.on_wait) if si and si.on_wait else []
            if len(w) > limit:
                extra, keep = w[:-limit], w[-limit:]
                for i in range(0, len(extra), limit):
                    ev = mybir.InstEventSemaphore(name=f"wsplit_{nid}", ins=[], outs=[])
                    nid += 1
                    ev.engine = ins.engine
                    ev.sync_info = mybir.SyncInfo(
                        on_wait=extra[i : i + limit], on_update=[]
                    )
                    out.append(ev)
                si.on_wait = keep
                changed = True
            out.append(ins)
        if changed:
            bb.instructions = out


def _build():
    nc = bass.Bass()
    x_ext = nc.declare_dram_parameter("x", [NS, DIM], F16, isOutput=False)
    w18_ext = nc.declare_dram_parameter("w18", [DIM, HIDDEN], FP8, isOutput=False)
    w1h_ext = nc.declare_dram_parameter("w1h", [DIM, HIDDEN], F16, isOutput=False)
    w2b_ext = nc.declare_dram_parameter("w2b", [HIDDEN, OUT_DIM], BF, isOutput=False)
    w2h_ext = nc.declare_dram_parameter("w2h", [HIDDEN, OUT_DIM], F16, isOutput=False)
    b1_ext = nc.declare_dram_parameter("b1", [HIDDEN, 1], F32, isOutput=False)
    b2_ext = nc.declare_dram_parameter("b2", [OUT_DIM, 1], F32, isOutput=False)
    y_ext = nc.declare_dram_parameter("y", [NS, OUT_DIM], F32, isOutput=True)

    with _TC(nc) as tc:
        with (
            tc.tile_pool(name="state", bufs=1) as st,
            tc.tile_pool(name="sq", bufs=2) as sqp,
            tc.tile_pool(name="xtp", bufs=2) as xtp,
            tc.tile_pool(name="stg", bufs=4) as stg,
            tc.tile_pool(name="ps", bufs=8, space="PSUM") as ps,
            tc.tile_pool(name="dram", bufs=2, space="DRAM") as dram,
        ):
            # persistent state (fp16, one tile so the XBAR can read 4-block
            # chunks and the final MLP needs no extra copy)
            xrall = st.tile([P, RT, DIM], F16, name="xrall", tag="xrall")
            xball = st.tile([P, RT, DIM], FP8, name="xball", tag="xball")
            xt8 = st.tile([P, RT, DK, P], FP8, name="xt8", tag="xt8")
            g8 = st.tile([P, DK, DIM], FP8, name="g8", tag="g8")
            et = st.tile([OUT_DIM, NS], BF, name="et", tag="et")
            erb = st.tile([P, RT, OUT_DIM], BF, name="erb", tag="erb")
            er16 = st.tile([P, RT, OUT_DIM], BF, name="er16", tag="er16")
            er8 = st.tile([P, RT, OUT_DIM], FP8, name="er8", tag="er8")
            etsb = st.tile([OUT_DIM, RT, P], BF, name="etsb", tag="etsb")
            ets8 = st.tile([OUT_DIM, RT, P], FP8, name="ets8", tag="ets8")
            ptx8 = st.tile([OUT_DIM, DIM], FP8, name="ptx8", tag="ptx8")
            gb = st.tile([P, DK, 512], BF, name="gb", tag="gb")
            gb2 = st.tile([P, DK, 512], BF, name="gb2", tag="gb2")
            ptxb = st.tile([OUT_DIM, DIM], BF, name="ptxb", tag="ptxb")
            a1 = st.tile([HIDDEN, NS], BF, name="a1", tag="a1")
            w18 = st.tile([P, DK, HIDDEN], FP8, name="w18", tag="w18")
            w1h = st.tile([P, DK, HIDDEN], F16, name="w1h", tag="w1h")
            w2b = st.tile([HIDDEN, OUT_DIM], BF, name="w2b", tag="w2b")
            w2h = st.tile([HIDDEN, OUT_DIM], F16, name="w2h", tag="w2h")
            b1 = st.tile([HIDDEN, 1], F32, name="b1", tag="b1")
            b2 = st.tile([OUT_DIM, 1], F32, name="b2", tag="b2")
            yr = st.tile([P, RT, OUT_DIM], F32, name="yr", tag="yr")
            n2 = st.tile([P, RT], F32, name="n2", tag="n2")
            sd = st.tile([P, RT], F32, name="sd", tag="sd")
            inv = st.tile([P, RT], F32, name="inv", tag="inv")
            invs = st.tile([P, RT], F32, name="invs", tag="invs")
            srow = st.tile([P, RT], F32, name="srow", tag="srow")
            srow64 = st.tile([P, RT], F32, name="srow64", tag="srow64")
            sinv64 = st.tile([P, RT], F32, name="sinv64", tag="sinv64")

            # fp16 views for the final-MLP path (reuse in-loop tiles)
            a1h = a1.bitcast(F16)
            yth = et.bitcast(F16)
            yrb = erb.bitcast(F16)

            # weight + state loads
            for i in range(RT):
                nc.gpsimd.dma_start(
                    xrall[:, i, :], x_ext[i * P : (i + 1) * P, :]
                )
            nc.gpsimd.dma_start(
                w18[:, :, :], w18_ext.rearrange("(k p) h -> p k h", p=P)
            )
            nc.gpsimd.dma_start(
                w1h[:, :, :], w1h_ext.rearrange("(k p) h -> p k h", p=P)
            )
            nc.gpsimd.dma_start(w2b[:], w2b_ext[:, :])
            nc.gpsimd.dma_start(w2h[:], w2h_ext[:, :])
            nc.gpsimd.dma_start(b1[:], b1_ext[:, :])
            nc.gpsimd.dma_start(b2[:], b2_ext[:, :])

            def norm_tail(i):
                # per-block square; per-quartet stats + in-place normalize
                # (fp16, DVE 2x) + xball = fp8(S * Xn) on ACT
                sq = sqp.tile([P, DIM], F16, name="sq", tag="sq")
                nc.scalar.activation(
                    sq[:], xrall[:, i, :], AF.Square, accum_out=n2[:, i : i + 1]
                )
                if i % 4 == 3:
                    j = i - 3
                    qs = slice(j, j + 4)
                    nc.scalar.sqrt(sd[:, qs], n2[:, qs])
                    nc.vector.reciprocal(inv[:, qs], sd[:, qs])
                    nc.vector.tensor_scalar_mul(invs[:, qs], inv[:, qs], S)
                    for b in range(j, j + 4):
                        nc.vector.tensor_scalar_mul(
                            xball[:, b, :], xrall[:, b, :], invs[:, b : b + 1]
                        )
                        nc.scalar.activation(
                            xrall[:, b, :], xrall[:, b, :], AF.Copy,
                            scale=inv[:, b : b + 1],
                        )

            def dma_t_chunks():
                # Xn (fp16) -> transposed [p, i, k, r] -> xt8 = fp8(S * XnT)
                for j in range(4):
                    tb = xtp.tile([P, 4, DK, P], F16, name="tb", tag="tb")
                    nc.sync.dma_start_transpose(
                        tb[:, :, :, :], xrall[:, 4 * j : 4 * (j + 1), :]
                    )
                    nc.vector.tensor_scalar_mul(
                        xt8[:, 4 * j : 4 * (j + 1), :, :], tb[:, :, :, :], S
                    )

            def phase_gram0(arin):
                # partial (S*Xn).T @ (S*Xn) cols 0:512, row-pair OUTER loop:
                # each pair's MMs fire as soon as its blocks are normalized,
                # overlapping the previous iteration's norm tail.
                pgs = [
                    ps.tile([P, 512], F32, name="ps", tag="ps") for _ in range(DK)
                ]
                for ip in range(RT // 2):
                    for m in range(DK):
                        nc.tensor.matmul(
                            pgs[m][:],
                            xball[:, 2 * ip : 2 * ip + 2, m * P : (m + 1) * P],
                            xball[:, 2 * ip : 2 * ip + 2, 0:512],
                            start=(ip == 0),
                            stop=(ip == RT // 2 - 1),
                            perf_mode=DRM,
                        )
                for m in range(DK):
                    gs = stg.tile([P, 512], ARDT, name="gs", tag="gs")
                    nc.scalar.activation(
                        gs[:], pgs[m][:], AF.Copy, scale=1.0 / (S * S)
                    )
                    nc.gpsimd.dma_start(arin[m * P : (m + 1) * P, :], gs[:])

            def phase_gram1(arin):
                # cols 512:1024, m-outer (norm tail long done by now)
                for m in range(DK):
                    pg = ps.tile([P, 512], F32, name="ps", tag="ps")
                    for ip in range(RT // 2):
                        nc.tensor.matmul(
                            pg[:],
                            xball[:, 2 * ip : 2 * ip + 2, m * P : (m + 1) * P],
                            xball[:, 2 * ip : 2 * ip + 2, 512:1024],
                            start=(ip == 0),
                            stop=(ip == RT // 2 - 1),
                            perf_mode=DRM,
                        )
                    gs = stg.tile([P, 512], ARDT, name="gs", tag="gs")
                    nc.scalar.activation(
                        gs[:], pg[:], AF.Copy, scale=1.0 / (S * S)
                    )
                    nc.gpsimd.dma_start(arin[m * P : (m + 1) * P, :], gs[:])

            def phase_mlp():
                # a1 = relu(Xn@W1 + b1).T ; et = exp(.@W2 + b2).T
                # (DoubleRow rhs must be 3D [K,2,N]: per 128-row block into
                # column regions of the [16,512] psum)
                for q in range(4):
                    sl = slice(q * 512, (q + 1) * 512)
                    pa = ps.tile([HIDDEN, 512], F32, name="ps", tag="ps")
                    for il in range(4):
                        i = 4 * q + il
                        for kp in range(DK // 2):
                            nc.tensor.matmul(
                                pa[:, il * P : (il + 1) * P],
                                w18[:, 2 * kp : 2 * kp + 2, :],
                                xt8[:, i, 2 * kp : 2 * kp + 2, :],
                                start=(kp == 0),
                                stop=(kp == DK // 2 - 1),
                                perf_mode=DRM,
                            )
                    nc.scalar.activation(
                        a1[:, sl], pa[:], AF.Relu, bias=b1[:], scale=1.0 / S
                    )
                    pb = ps.tile([OUT_DIM, 512], F32, name="ps", tag="ps")
                    nc.tensor.matmul(pb[:], w2b[:], a1[:, sl])
                    nc.scalar.activation(et[:, sl], pb[:], AF.Exp, bias=b2[:])

            def phase_p():
                # et -> erb (rows) via DMA xbar in two halves (pipelines with
                # the MLP); row sums; er16 = 64*P rows -> er8 fp8;
                # er16 -> etsb -> ets8 = 4*P.T
                for hf in range(2):
                    bs = slice(8 * hf, 8 * (hf + 1))
                    nc.sync.dma_start_transpose(
                        erb[:, bs, :], et[:, hf * 1024 : (hf + 1) * 1024]
                    )
                    nc.vector.tensor_reduce(
                        srow[:, bs], erb[:, bs, :], mybir.AxisListType.X, ALU.add
                    )
                    nc.vector.tensor_scalar_mul(
                        srow64[:, bs], srow[:, bs], 1.0 / 64.0
                    )
                    nc.vector.reciprocal(sinv64[:, bs], srow64[:, bs])
                    for i in range(8 * hf, 8 * (hf + 1)):
                        nc.vector.tensor_scalar_mul(
                            er16[:, i, :], erb[:, i, :], sinv64[:, i : i + 1]
                        )
                    nc.vector.tensor_copy(er8[:, bs, :], er16[:, bs, :])
                nc.sync.dma_start_transpose(etsb[:, :, :], er16[:, :, :])
                nc.vector.tensor_scalar_mul(
                    ets8[:, :, :], etsb[:, :, :], 1.0 / 16.0
                )

            def phase_ptx(arin):
                # partial P.T @ Xn in fp8 DoubleRow: psum = (64P).T@(4Xn);
                # staging = -psum/256 = -PtX_partial (sign folded here)
                for h in range(2):
                    pp = ps.tile([OUT_DIM, 512], F32, name="ps", tag="ps")
                    for ip in range(RT // 2):
                        nc.tensor.matmul(
                            pp[:],
                            er8[:, 2 * ip : 2 * ip + 2, :],
                            xball[:, 2 * ip : 2 * ip + 2, h * 512 : (h + 1) * 512],
                            start=(ip == 0),
                            stop=(ip == RT // 2 - 1),
                            perf_mode=DRM,
                        )
                    pps = stg.tile([OUT_DIM, 512], ARDT, name="pps", tag="gs")
                    nc.scalar.activation(
                        pps[:], pp[:], AF.Copy, scale=-1.0 / (64.0 * S)
                    )
                    nc.gpsimd.dma_start(arin[:, h * 512 : (h + 1) * 512], pps[:])

            def phase_j(h, tail=None):
                # state update cols h*512..: psum = 4*Xn@G - 4*P@PtX;
                # xsl = psum*(-GAMMA/4) + Xn   (fp16 state in/out)
                for i in range(RT):
                    pm = ps.tile([P, 512], F32, name="ps", tag="ps")
                    for kp in range(DK // 2):
                        nc.tensor.matmul(
                            pm[:],
                            xt8[:, i, 2 * kp : 2 * kp + 2, :],
                            g8[:, 2 * kp : 2 * kp + 2, h * 512 : (h + 1) * 512],
                            start=(kp == 0),
                            stop=False,
                            perf_mode=DRM,
                        )
                    nc.tensor.matmul(
                        pm[:],
                        ets8[:, i, :],
                        ptx8[:, h * 512 : (h + 1) * 512],
                        start=False,
                        stop=True,
                    )
                    xsl = xrall[:, i, h * 512 : (h + 1) * 512]
                    nc.vector.scalar_tensor_tensor(
                        xsl, pm[:], CG, xsl, ALU.mult, ALU.add
                    )
                    if tail is not None:
                        tail(i)

            # prologue: normalize + produce xball/xt8 for iteration 0
            for i in range(RT):
                norm_tail(i)
            dma_t_chunks()

            rg = [list(range(CORES))]
            for it in range(DEPTH):
                arin_a = dram.tile([1024, 512], ARDT, name="arin_a", tag="arin_a")
                arout_a = dram.tile([1024, 512], ARDT, name="arout_a", tag="arout_a", addr_space="Shared")
                arin_b = dram.tile([1024, 512], ARDT, name="arin_b", tag="arin_b")
                arout_b = dram.tile([1024, 512], ARDT, name="arout_b", tag="arout_b", addr_space="Shared")
                arin_c = dram.tile([OUT_DIM, DIM], ARDT, name="arin_c", tag="arin_c")
                arout_c = dram.tile([OUT_DIM, DIM], ARDT, name="arout_c", tag="arout_c", addr_space="Shared")

                if it > 0:
                    dma_t_chunks()

                phase_gram0(arin_a)
                nc.gpsimd.collective_compute(
                    "AllReduce", ALU.add,
                    ins=[arin_a.opt()], outs=[arout_a.opt()], replica_groups=rg,
                )
                phase_mlp()
                phase_p()
                phase_ptx(arin_c)
                nc.gpsimd.collective_compute(
                    "AllReduce", ALU.add,
                    ins=[arin_c.opt()], outs=[arout_c.opt()], replica_groups=rg,
                )
                phase_gram1(arin_b)
                nc.gpsimd.collective_compute(
                    "AllReduce", ALU.add,
                    ins=[arin_b.opt()], outs=[arout_b.opt()], replica_groups=rg,
                )

                # land G cols 0:512 + (-PtX) into the fp8 operands
                if USE_FP8_AR:
                    nc.sync.dma_start(
                        g8[:, :, 0:512], arout_a.rearrange("(k p) c -> p k c", p=P)
                    )
                    nc.sync.dma_start(ptx8[:, :], arout_c[:, :])
                else:
                    nc.sync.dma_start(
                        gb[:, :, :], arout_a.rearrange("(k p) c -> p k c", p=P)
                    )
                    nc.vector.tensor_copy(g8[:, :, 0:512], gb[:, :, :])
                    nc.sync.dma_start(ptxb[:, :], arout_c[:, :])
                    nc.vector.tensor_copy(ptx8[:, :], ptxb[:, :])

                phase_j(0)

                # land G cols 512:1024 while J(0) runs
                if USE_FP8_AR:
                    nc.sync.dma_start(
                        g8[:, :, 512:1024], arout_b.rearrange("(k p) c -> p k c", p=P)
                    )
                else:
                    nc.sync.dma_start(
                        gb2[:, :, :], arout_b.rearrange("(k p) c -> p k c", p=P)
                    )
                    nc.vector.tensor_copy(g8[:, :, 512:1024], gb2[:, :, :])

                tail = norm_tail if it < DEPTH - 1 else None
                phase_j(1, tail=tail)

            # final MLP in fp16 straight off the (raw) fp16 state
            for q in range(4):
                tb = xtp.tile([P, 4, DK, P], F16, name="tb", tag="tb")
                nc.sync.dma_start_transpose(
                    tb[:, :, :, :], xrall[:, 4 * q : 4 * (q + 1), :]
                )
                sl = slice(q * 512, (q + 1) * 512)
                pa = ps.tile([HIDDEN, 512], F32, name="ps", tag="ps")
                for k in range(DK):
                    nc.tensor.matmul(
                        pa[:],
                        w1h[:, k, :],
                        tb[:, :, k, :],
                        start=(k == 0),
                        stop=(k == DK - 1),
                    )
                nc.scalar.activation(a1h[:, sl], pa[:], AF.Relu, bias=b1[:])
                pb = ps.tile([OUT_DIM, 512], F32, name="ps", tag="ps")
                nc.tensor.matmul(pb[:], w2h[:], a1h[:, sl])
                nc.scalar.activation(yth[:, sl], pb[:], AF.Identity, bias=b2[:])
            # transpose Y.T -> rows (DMA xbar), cast fp32, store
            nc.sync.dma_start_transpose(yrb[:, :, :], yth[:, :])
            nc.vector.tensor_copy(yr[:, :, :], yrb[:, :, :])
            nc.gpsimd.dma_start(
                y_ext.rearrange("(i p) o -> p i o", p=P), yr[:, :, :]
            )

    _split_waits(nc)
    return nc


_NC = None


def _get_nc():
    global _NC
    if _NC is None:
        _NC = _build()
    return _NC


def _in_maps(X, W1, b1, W2, b2):
    X = np.asarray(X, dtype=np.float32).astype(np.float16)
    W1 = np.asarray(W1, dtype=np.float32)
    W2 = np.asarray(W2, dtype=np.float32)
    b1 = np.asarray(b1, dtype=np.float32).reshape(HIDDEN, 1)
    b2 = np.asarray(b2, dtype=np.float32).reshape(OUT_DIM, 1)
    w18 = W1.astype(ml_dtypes.float8_e4m3fn)
    w1h = W1.astype(np.float16)
    w2b = W2.astype(ml_dtypes.bfloat16)
    w2h = W2.astype(np.float16)
    return [
        {
            "x": np.ascontiguousarray(X[c * NS : (c + 1) * NS]),
            "w18": w18,
            "w1h": w1h,
            "w2b": w2b,
            "w2h": w2h,
            "b1": b1,
            "b2": b2,
        }
        for c in range(CORES)
    ]


def run(X, W1, b1, W2, b2, **kwargs):
    nc = _get_nc()
    res = run_bass_kernel_spmd(nc, _in_maps(X, W1, b1, W2, b2), list(range(CORES)), **kwargs)
    out = np.concatenate([res.results[c]["y"] for c in range(CORES)], axis=0)
    return out, res


def kernel(X, W1, b1, W2, b2):
    out, _ = run(X, W1, b1, W2, b2)
    return out
